# revision 4
# baseline (speedup 1.0000x reference)
"""Trainium2 Bass kernel for nn_Decoder (2-layer GRU decoder, weight-tied vocab projection).

Strategy (8 NeuronCores, SPMD):
  - Tensor-parallel recurrence: each core computes a 128-row slice of every GRU
    gate (H=1024 -> 8 x 128). Per superstep, one fused AllGather exchanges the
    new h0/h1 slices (bf16) across cores.
  - Gate preactivations are built entirely in PSUM by fused matmul groups
    (w_hh @ h  +  w_ih @ x  + bias-outer-product), fp32 accumulate.
  - The h-state for the "z*h_prev" path is kept in fp32 locally (h_own); only
    matmul operands are bf16.
  - Vocab-tied projection: embedding^T is sharded 4000 cols/core; the MLP is
    computed replicated (cheap) and logits are vocab-sharded.
Output: (B=16, S=128, V=32000) fp32, assembled host-side.
"""
import os
import numpy as np
import ml_dtypes

V, E, H, L = 32000, 512, 1024, 2
B, S = 16, 128
N = 8                  # cores
HS = H // N            # 128 rows of h per core
VS = V // N            # 4000 vocab cols per core
TB = B * S             # 2048 (t,b) rows
STEPS = int(os.environ.get("K_STEPS", str(S)))  # reduced for smoke testing

_cache = {}


def _build():
    import concourse.bass as bass
    import concourse.bacc as bacc
    import concourse.mybir as mybir
    import concourse.tile as tile
    from concourse.masks import make_identity

    fp32 = mybir.dt.float32
    bf16 = mybir.dt.bfloat16
    i32 = mybir.dt.int32
    T = STEPS
    NT = TB // 128 if T == S else (T * B) // 128   # number of 128-row (t,b) tiles
    RW = 64            # h0 ring: 4 slots x 16 cols per k-tile

    nc = bacc.Bacc("TRN2", num_devices=N, target_bir_lowering=False)

    # ---- DRAM I/O ----
    emb = nc.dram_tensor("emb", [V, E], fp32, kind="ExternalInput")
    idx = nc.dram_tensor("idx", [B * T // 128 * 8 if False else (T * B // 128), 128], i32, kind="ExternalInput")
    h0f = nc.dram_tensor("h0f", [H, B], bf16, kind="ExternalInput")
    h1f = nc.dram_tensor("h1f", [H, B], bf16, kind="ExternalInput")
    h0o = nc.dram_tensor("h0o", [HS, B], fp32, kind="ExternalInput")
    h1o = nc.dram_tensor("h1o", [HS, B], fp32, kind="ExternalInput")
    wih0 = nc.dram_tensor("wih0", [E, 3 * HS], bf16, kind="ExternalInput")
    whh0 = nc.dram_tensor("whh0", [H, 3 * HS], bf16, kind="ExternalInput")
    wih1 = nc.dram_tensor("wih1", [H, 3 * HS], bf16, kind="ExternalInput")
    whh1 = nc.dram_tensor("whh1", [H, 3 * HS], bf16, kind="ExternalInput")
    ba0 = nc.dram_tensor("ba0", [1, 3 * HS], bf16, kind="ExternalInput")
    bb0 = nc.dram_tensor("bb0", [1, HS], bf16, kind="ExternalInput")
    ba1 = nc.dram_tensor("ba1", [1, 3 * HS], bf16, kind="ExternalInput")
    bb1 = nc.dram_tensor("bb1", [1, HS], bf16, kind="ExternalInput")
    w1t = nc.dram_tensor("w1t", [H, H], bf16, kind="ExternalInput")
    b1c = nc.dram_tensor("b1c", [128, 8], fp32, kind="ExternalInput")
    w2t = nc.dram_tensor("w2t", [H, E], bf16, kind="ExternalInput")
    b2c = nc.dram_tensor("b2c", [128, 4], fp32, kind="ExternalInput")
    embts = nc.dram_tensor("embts", [E, VS], bf16, kind="ExternalInput")
    bgen = nc.dram_tensor("bgen", [1, VS], bf16, kind="ExternalInput")
    out = nc.dram_tensor("out", [T * B, VS], fp32, kind="ExternalOutput")

    with tile.TileContext(nc) as tc:
        with (
            tc.tile_pool(name="wp", bufs=1) as wp,
            tc.tile_pool(name="state", bufs=1) as st,
            tc.tile_pool(name="dram", bufs=4, space="DRAM") as dr,
        ):
            # ---- resident weights ----
            def load3d(name, src, kdim, cols):
                # src (kdim*128, cols) -> sbuf (128, kdim*cols), k-major blocks
                tl = wp.tile([128, kdim * cols], bf16, tag=name, name=name)
                nc.sync.dma_start(
                    tl[:].rearrange("p (k c) -> p k c", k=kdim),
                    src[:].rearrange("(k p) c -> p k c", p=128))
                return tl

            wih0_sb = load3d("wih0_sb", wih0, 4, 3 * HS)
            whh0_sb = load3d("whh0_sb", whh0, 8, 3 * HS)
            wih1_sb = load3d("wih1_sb", wih1, 8, 3 * HS)
            whh1_sb = load3d("whh1_sb", whh1, 8, 3 * HS)
            w1t_sb = load3d("w1t_sb", w1t, 8, H)
            w2t_sb = load3d("w2t_sb", w2t, 8, E)
            embts_sb = load3d("embts_sb", embts, 4, VS)
            ba0_sb = wp.tile([128, 3 * HS], bf16, tag="ba0s", name="ba0s")
            nc.sync.dma_start(ba0_sb[0:1, :], ba0[:])
            bb0_sb = wp.tile([128, HS], bf16, tag="bb0s", name="bb0s")
            nc.sync.dma_start(bb0_sb[0:1, :], bb0[:])
            ba1_sb = wp.tile([128, 3 * HS], bf16, tag="ba1s", name="ba1s")
            nc.sync.dma_start(ba1_sb[0:1, :], ba1[:])
            bb1_sb = wp.tile([128, HS], bf16, tag="bb1s", name="bb1s")
            nc.sync.dma_start(bb1_sb[0:1, :], bb1[:])
            b1_sb = wp.tile([128, 8], fp32, tag="b1s", name="b1s")
            nc.sync.dma_start(b1_sb[:], b1c[:])
            b2_sb = wp.tile([128, 4], fp32, tag="b2s", name="b2s")
            nc.sync.dma_start(b2_sb[:], b2c[:])
            bgen_sb = wp.tile([128, VS], bf16, tag="bgens", name="bgens")
            nc.sync.dma_start(bgen_sb[0:1, :], bgen[:])
            ones_sb = wp.tile([128, 128], bf16, tag="ones", name="ones")
            nc.vector.memset(ones_sb[:], 1.0)

            # ---- state ----
            XT = st.tile([128, 4 * (T * B)], bf16, tag="XT", name="XT")        # x^T, 4 E-tiles
            ring0 = st.tile([128, 8 * RW], bf16, tag="ring0", name="ring0")    # h0 ring, 8 k-tiles
            H1T = st.tile([128, 8 * (16 * (T + 1))], bf16, tag="H1T", name="H1T")
            h_own0 = [st.tile([128, B], fp32, tag=f"ho0_{p}", name=f"ho0_{p}") for p in range(2)]
            h_own1 = [st.tile([128, B], fp32, tag=f"ho1_{p}", name=f"ho1_{p}") for p in range(2)]
            HW1 = 16 * (T + 1)

            # init h state
            nc.sync.dma_start(
                ring0[:].rearrange("p (k c) -> p k c", k=8)[:, :, 0:16],
                h0f[:].rearrange("(k p) c -> p k c", p=128))
            nc.sync.dma_start(
                H1T[:].rearrange("p (k c) -> p k c", k=8)[:, :, 0:16],
                h1f[:].rearrange("(k p) c -> p k c", p=128))
            nc.sync.dma_start(h_own0[0][:], h0o[:])
            nc.sync.dma_start(h_own1[0][:], h1o[:])

            # ---- P1: gather + transpose x ----
            with (
                tc.tile_pool(name="gp", bufs=2) as gp,
                tc.tile_pool(name="gps", bufs=2, space="PSUM") as gpp,
            ):
                idx_sb = gp.tile([128, NT], i32, tag="idx", name="idx")
                nc.sync.dma_start(idx_sb[:], idx[:].rearrange("a b -> b a"))
                ident = gp.tile([128, 128], fp32, tag="ident", name="ident")
                make_identity(nc, ident[:])
                for i in range(NT):
                    xg = gp.tile([128, E], fp32, tag="xg", name=f"xg{i}", bufs=2)
                    nc.gpsimd.indirect_dma_start(
                        out=xg[:], out_offset=None, in_=emb[:],
                        in_offset=bass.IndirectOffsetOnAxis(ap=idx_sb[:, i:i + 1], axis=0))
                    for e in range(4):
                        pt = gpp.tile([128, 128], fp32, tag="pt", name=f"pt{i}_{e}")
                        nc.tensor.transpose(pt[:], xg[:, 128 * e:128 * (e + 1)], ident[:])
                        nc.vector.tensor_copy(
                            XT[:, (T * B) * e + 128 * i: (T * B) * e + 128 * (i + 1)], pt[:])

            # ---- P2: recurrence supersteps ----
            def gate_groups(ps, whh_sb, wih_sb, kdim_i, rhs_h_fn, rhs_x_fn, baS, bbS):
                # build r,z,i_n,h_n psum groups into ps(128,64)
                for gi, (c0, gc) in enumerate([(0, 0), (16, HS), (32, 2 * HS)]):
                    # r (c0=0), z (c0=16): h-part + x-part + bias; i_n (c0=32): x-part + bias
                    mms = []
                    if gi < 2:
                        for k in range(8):
                            mms.append((whh_sb[:, 3 * HS * k + gc:3 * HS * k + gc + 128], rhs_h_fn(k)))
                    for k in range(kdim_i):
                        mms.append((wih_sb[:, 3 * HS * k + gc:3 * HS * k + gc + 128], rhs_x_fn(k)))
                    mms.append((baS[0:1, gc:gc + 128], ones_sb[0:1, 0:B]))
                    for mi, (lt, rr) in enumerate(mms):
                        nc.tensor.matmul(ps[:, c0:c0 + 16], lt, rr,
                                         start=(mi == 0), stop=(mi == len(mms) - 1))
                # h_n group (cols 48:64)
                mms = [(whh_sb[:, 3 * HS * k + 2 * HS:3 * HS * k + 2 * HS + 128], rhs_h_fn(k))
                       for k in range(8)]
                mms.append((bbS[0:1, 0:HS], ones_sb[0:1, 0:B]))
                for mi, (lt, rr) in enumerate(mms):
                    nc.tensor.matmul(ps[:, 48:64], lt, rr,
                                     start=(mi == 0), stop=(mi == len(mms) - 1))

            def gates(ps, hprev, hnew, cc_dst, tag, t):
                rz = st.tile([128, 32], fp32, tag=f"rz{tag}", name=f"rz{tag}_{t}", bufs=2)
                nc.scalar.activation(rz[:], ps[:, 0:32], mybir.ActivationFunctionType.Sigmoid)
                t1 = st.tile([128, B], fp32, tag=f"t1{tag}", name=f"t1{tag}_{t}", bufs=2)
                nc.vector.tensor_tensor(out=t1[:], in0=ps[:, 48:64], in1=rz[:, 0:16],
                                        op=mybir.AluOpType.mult)
                nc.vector.tensor_tensor(out=t1[:], in0=t1[:], in1=ps[:, 32:48],
                                        op=mybir.AluOpType.add)
                nsb = st.tile([128, B], fp32, tag=f"n{tag}", name=f"n{tag}_{t}", bufs=2)
                nc.scalar.activation(nsb[:], t1[:], mybir.ActivationFunctionType.Tanh)
                a = st.tile([128, B], fp32, tag=f"a{tag}", name=f"a{tag}_{t}", bufs=2)
                nc.vector.tensor_tensor(out=a[:], in0=hprev[:], in1=nsb[:],
                                        op=mybir.AluOpType.subtract)
                nc.vector.tensor_tensor(out=a[:], in0=a[:], in1=rz[:, 16:32],
                                        op=mybir.AluOpType.mult)
                nc.vector.tensor_tensor(out=hnew[:], in0=a[:], in1=nsb[:],
                                        op=mybir.AluOpType.add)
                nc.vector.tensor_copy(cc_dst, hnew[:])

            ring3 = ring0[:].rearrange("p (k c) -> p k c", k=8)
            h1t3 = H1T[:].rearrange("p (k c) -> p k c", k=8)

            psp_cm = tc.tile_pool(name="ps", bufs=2, space="PSUM")
            psp = psp_cm.__enter__()
            for t in range(1, T + 2):
                cc_src = st.tile([128, 32], bf16, tag="cc_src", name=f"cc_src{t}", bufs=2)
                if t <= T:
                    # layer 0: h0_t
                    ps0 = psp.tile([128, 64], fp32, tag="ps0", name=f"ps0_{t}", bufs=2)
                    so = 16 * ((t - 1) % 4)
                    gate_groups(
                        ps0, whh0_sb, wih0_sb, 4,
                        lambda k: ring0[:, RW * k + so:RW * k + so + 16],
                        lambda e: XT[:, (T * B) * e + B * (t - 1):(T * B) * e + B * t],
                        ba0_sb, bb0_sb)
                    gates(ps0, h_own0[(t - 1) % 2], h_own0[t % 2], cc_src[:, 0:16], "L0", t)
                if t >= 2:
                    # layer 1: h1_{t-1}
                    ps1 = psp.tile([128, 64], fp32, tag="ps1", name=f"ps1_{t}", bufs=2)
                    so1 = 16 * ((t - 1) % 4)
                    gate_groups(
                        ps1, whh1_sb, wih1_sb, 8,
                        lambda k: H1T[:, HW1 * k + 16 * (t - 2):HW1 * k + 16 * (t - 1)],
                        lambda k: ring0[:, RW * k + so1:RW * k + so1 + 16],
                        ba1_sb, bb1_sb)
                    gates(ps1, h_own1[(t - 2) % 2], h_own1[(t - 1) % 2], cc_src[:, 16:32], "L1", t)
                else:
                    nc.vector.memset(cc_src[:, 16:32], 0.0)
                if t == T + 1:
                    nc.vector.memset(cc_src[:, 0:16], 0.0)

                ccin = dr.tile([128, 32], bf16, tag="ccin", name=f"ccin{t}", bufs=2)
                ccout = dr.tile([128 * N, 32], bf16, tag="ccout", name=f"ccout{t}",
                                bufs=2, addr_space="Shared")
                nc.sync.dma_start(ccin[:], cc_src[:])
                nc.gpsimd.collective_compute(
                    "AllGather", mybir.AluOpType.bypass,
                    replica_groups=[list(range(N))],
                    ins=[ccin[:]], outs=[ccout[:]])
                if t <= T:
                    nc.sync.dma_start(
                        ring3[:, :, 16 * (t % 4):16 * (t % 4) + 16],
                        ccout[:, 0:16].rearrange("(k p) c -> p k c", p=128))
                if t >= 2:
                    nc.sync.dma_start(
                        h1t3[:, :, 16 * (t - 1):16 * t],
                        ccout[:, 16:32].rearrange("(k p) c -> p k c", p=128))

            psp_cm.__exit__(None, None, None)

            # ---- P5: MLP + logits ----
            with (
                tc.tile_pool(name="mp", bufs=1) as mp,
                tc.tile_pool(name="mps", bufs=4, space="PSUM") as mpp,
            ):
                RT = mp.tile([128, 8 * (T * B)], bf16, tag="RT", name="RT")
                for m in range(8):
                    for nchs in range(0, T * B, 512):
                        nw = min(512, T * B - nchs)
                        pr = mpp.tile([128, nw], fp32, tag="pr", name=f"pr{m}_{nchs}", bufs=2)
                        for k in range(8):
                            nc.tensor.matmul(
                                pr[:], w1t_sb[:, H * k + 128 * m:H * k + 128 * (m + 1)],
                                H1T[:, HW1 * k + 16 + nchs:HW1 * k + 16 + nchs + nw],
                                start=(k == 0), stop=(k == 7))
                        nc.scalar.activation(
                            RT[:, (T * B) * m + nchs:(T * B) * m + nchs + nw], pr[:],
                            mybir.ActivationFunctionType.Relu, bias=b1_sb[:, m:m + 1], scale=1.0)
                OUTT = mp.tile([128, 4 * (T * B)], bf16, tag="OUTT", name="OUTT")
                for m in range(4):
                    for nchs in range(0, T * B, 512):
                        nw = min(512, T * B - nchs)
                        po = mpp.tile([128, nw], fp32, tag="po", name=f"po{m}_{nchs}", bufs=2)
                        for k in range(8):
                            nc.tensor.matmul(
                                po[:], w2t_sb[:, E * k + 128 * m:E * k + 128 * (m + 1)],
                                RT[:, (T * B) * k + nchs:(T * B) * k + nchs + nw],
                                start=(k == 0), stop=(k == 7))
                        nc.scalar.activation(
                            OUTT[:, (T * B) * m + nchs:(T * B) * m + nchs + nw], po[:],
                            mybir.ActivationFunctionType.Identity, bias=b2_sb[:, m:m + 1], scale=1.0)
                for j in range(NT):
                    lsb = mp.tile([128, VS], fp32, tag="lsb", name=f"lsb{j}", bufs=1)
                    for nchs in range(0, VS, 500):
                        pl = mpp.tile([128, 500], fp32, tag="pl", name=f"pl{j}_{nchs}", bufs=4)
                        for e in range(4):
                            nc.tensor.matmul(
                                pl[:], OUTT[:, (T * B) * e + 128 * j:(T * B) * e + 128 * (j + 1)],
                                embts_sb[:, VS * e + nchs:VS * e + nchs + 500],
                                start=(e == 0), stop=False)
                        nc.tensor.matmul(
                            pl[:], ones_sb[0:1, :], bgen_sb[0:1, nchs:nchs + 500],
                            start=False, stop=True)
                        nc.vector.tensor_copy(lsb[:, nchs:nchs + 500], pl[:])
                    nc.sync.dma_start(out[128 * j:128 * (j + 1), :], lsb[:])

    nc.finalize()
    return nc


def _prep_inputs(hidden, trg, embedding, w_ih0, w_hh0, b_ih0, b_hh0,
                 w_ih1, w_hh1, b_ih1, b_hh1, w1, b1, w2, b2, b_gen):
    bf = ml_dtypes.bfloat16
    T = STEPS
    f32 = np.float32
    hidden = np.asarray(hidden, f32)
    trg = np.asarray(trg)
    embedding = np.asarray(embedding, f32)
    in_maps = []
    # (t,b) index order
    idx_full = np.asarray(trg.T[:T], np.int32).reshape(-1)          # (T*B,)
    idx_tiles = idx_full.reshape(-1, 128).astype(np.int32)          # (NT,128)

    def gslice(wT, r):
        # wT (K, 3H) -> (K, 384) slice of each gate for core r
        cols = np.concatenate([np.arange(HS) + g * H + r * HS for g in range(3)])
        return np.ascontiguousarray(wT[:, cols])

    for r in range(N):
        sl = slice(r * HS, (r + 1) * HS)
        ba0_ = (b_ih0 + b_hh0).astype(f32)
        ba0v = np.concatenate([ba0_[0 * H + r * HS:0 * H + (r + 1) * HS],
                               ba0_[1 * H + r * HS:1 * H + (r + 1) * HS],
                               np.asarray(b_ih0, f32)[2 * H + r * HS:2 * H + (r + 1) * HS]])
        bb0v = np.asarray(b_hh0, f32)[2 * H + r * HS:2 * H + (r + 1) * HS]
        ba1_ = (b_ih1 + b_hh1).astype(f32)
        ba1v = np.concatenate([ba1_[0 * H + r * HS:0 * H + (r + 1) * HS],
                               ba1_[1 * H + r * HS:1 * H + (r + 1) * HS],
                               np.asarray(b_ih1, f32)[2 * H + r * HS:2 * H + (r + 1) * HS]])
        bb1v = np.asarray(b_hh1, f32)[2 * H + r * HS:2 * H + (r + 1) * HS]
        in_maps.append({
            "emb": embedding,
            "idx": idx_tiles,
            "h0f": hidden[0].T.astype(bf),
            "h1f": hidden[1].T.astype(bf),
            "h0o": np.ascontiguousarray(hidden[0].T[sl]).astype(f32),
            "h1o": np.ascontiguousarray(hidden[1].T[sl]).astype(f32),
            "wih0": gslice(np.asarray(w_ih0, f32).T, r).astype(bf),
            "whh0": gslice(np.asarray(w_hh0, f32).T, r).astype(bf),
            "wih1": gslice(np.asarray(w_ih1, f32).T, r).astype(bf),
            "whh1": gslice(np.asarray(w_hh1, f32).T, r).astype(bf),
            "ba0": ba0v.reshape(1, -1).astype(bf),
            "bb0": bb0v.reshape(1, -1).astype(bf),
            "ba1": ba1v.reshape(1, -1).astype(bf),
            "bb1": bb1v.reshape(1, -1).astype(bf),
            "w1t": np.asarray(w1, f32).T.astype(bf),
            "b1c": np.asarray(b1, f32).reshape(8, 128).T.astype(f32),
            "w2t": np.asarray(w2, f32).T.astype(bf),
            "b2c": np.asarray(b2, f32).reshape(4, 128).T.astype(f32),
            "embts": np.ascontiguousarray(embedding.T[:, r * VS:(r + 1) * VS]).astype(bf),
            "bgen": np.asarray(b_gen, f32)[r * VS:(r + 1) * VS].reshape(1, -1).astype(bf),
        })
    return in_maps


def kernel(**inputs):
    from concourse.bass_utils import run_bass_kernel_spmd
    if "nc" not in _cache:
        _cache["nc"] = _build()
    nc = _cache["nc"]
    in_maps = _prep_inputs(**inputs)
    res = run_bass_kernel_spmd(nc, in_maps, core_ids=list(range(N)))
    T = STEPS
    outf = np.empty((B, T, V), np.float32)
    for r in range(N):
        lr = res.results[r]["out"].reshape(T, B, VS)
        outf[:, :, r * VS:(r + 1) * VS] = lr.transpose(1, 0, 2)
    return outf


# revision 7
# speedup vs baseline: 1.0113x; 1.0113x over previous
"""Trainium2 Bass kernel for nn_Decoder (2-layer GRU decoder, weight-tied vocab projection).

Strategy (8 NeuronCores, SPMD):
  - Tensor-parallel recurrence: each core computes a 128-row slice of every GRU
    gate (H=1024 -> 8 x 128). Per superstep, one fused AllGather exchanges the
    new h0/h1 slices (bf16) across cores.
  - Gate preactivations are built entirely in PSUM by fused matmul groups
    (w_hh @ h  +  w_ih @ x  + bias-outer-product), fp32 accumulate.
  - The h-state for the "z*h_prev" path is kept in fp32 locally (h_own); only
    matmul operands are bf16.
  - Vocab-tied projection: embedding^T is sharded 4000 cols/core; the MLP is
    computed replicated (cheap) and logits are vocab-sharded.
Output: (B=16, S=128, V=32000) fp32, assembled host-side.
"""
import os
import numpy as np
import ml_dtypes

V, E, H, L = 32000, 512, 1024, 2
B, S = 16, 128
N = 8                  # cores
HS = H // N            # 128 rows of h per core
VS = V // N            # 4000 vocab cols per core
TB = B * S             # 2048 (t,b) rows
STEPS = int(os.environ.get("K_STEPS", str(S)))  # reduced for smoke testing

_cache = {}


def _build():
    import concourse.bass as bass
    import concourse.bacc as bacc
    import concourse.mybir as mybir
    import concourse.tile as tile
    from concourse.masks import make_identity

    fp32 = mybir.dt.float32
    bf16 = mybir.dt.bfloat16
    i32 = mybir.dt.int32
    T = STEPS
    NT = TB // 128 if T == S else (T * B) // 128   # number of 128-row (t,b) tiles
    RW = 64            # h0 ring: 4 slots x 16 cols per k-tile

    nc = bacc.Bacc("TRN2", num_devices=N, target_bir_lowering=False)

    # ---- DRAM I/O ----
    emb = nc.dram_tensor("emb", [V, E], fp32, kind="ExternalInput")
    idx = nc.dram_tensor("idx", [B * T // 128 * 8 if False else (T * B // 128), 128], i32, kind="ExternalInput")
    h0f = nc.dram_tensor("h0f", [H, B], bf16, kind="ExternalInput")
    h1f = nc.dram_tensor("h1f", [H, B], bf16, kind="ExternalInput")
    h0o = nc.dram_tensor("h0o", [HS, B], fp32, kind="ExternalInput")
    h1o = nc.dram_tensor("h1o", [HS, B], fp32, kind="ExternalInput")
    wih0 = nc.dram_tensor("wih0", [E, 3 * HS], bf16, kind="ExternalInput")
    whh0 = nc.dram_tensor("whh0", [H, 3 * HS], bf16, kind="ExternalInput")
    wih1 = nc.dram_tensor("wih1", [H, 3 * HS], bf16, kind="ExternalInput")
    whh1 = nc.dram_tensor("whh1", [H, 3 * HS], bf16, kind="ExternalInput")
    ba0 = nc.dram_tensor("ba0", [1, 3 * HS], bf16, kind="ExternalInput")
    bb0 = nc.dram_tensor("bb0", [1, HS], bf16, kind="ExternalInput")
    ba1 = nc.dram_tensor("ba1", [1, 3 * HS], bf16, kind="ExternalInput")
    bb1 = nc.dram_tensor("bb1", [1, HS], bf16, kind="ExternalInput")
    w1t = nc.dram_tensor("w1t", [H, H], bf16, kind="ExternalInput")
    b1c = nc.dram_tensor("b1c", [128, 8], fp32, kind="ExternalInput")
    w2t = nc.dram_tensor("w2t", [H, E], bf16, kind="ExternalInput")
    b2c = nc.dram_tensor("b2c", [128, 4], fp32, kind="ExternalInput")
    embts = nc.dram_tensor("embts", [E, VS], bf16, kind="ExternalInput")
    bgen = nc.dram_tensor("bgen", [1, VS], bf16, kind="ExternalInput")
    out = nc.dram_tensor("out", [T * B, VS], fp32, kind="ExternalOutput")

    with tile.TileContext(nc) as tc:
        with (
            tc.tile_pool(name="wp", bufs=1) as wp,
            tc.tile_pool(name="state", bufs=1) as st,
            tc.tile_pool(name="dram", bufs=4, space="DRAM") as dr,
        ):
            # ---- resident weights ----
            def load3d(name, src, kdim, cols):
                # src (kdim*128, cols) -> sbuf (128, kdim*cols), k-major blocks
                tl = wp.tile([128, kdim * cols], bf16, tag=name, name=name)
                nc.sync.dma_start(
                    tl[:].rearrange("p (k c) -> p k c", k=kdim),
                    src[:].rearrange("(k p) c -> p k c", p=128))
                return tl

            wih0_sb = load3d("wih0_sb", wih0, 4, 3 * HS)
            whh0_sb = load3d("whh0_sb", whh0, 8, 3 * HS)
            wih1_sb = load3d("wih1_sb", wih1, 8, 3 * HS)
            whh1_sb = load3d("whh1_sb", whh1, 8, 3 * HS)
            w1t_sb = load3d("w1t_sb", w1t, 8, H)
            w2t_sb = load3d("w2t_sb", w2t, 8, E)
            embts_sb = load3d("embts_sb", embts, 4, VS)
            ba0_sb = wp.tile([128, 3 * HS], bf16, tag="ba0s", name="ba0s")
            nc.sync.dma_start(ba0_sb[0:1, :], ba0[:])
            bb0_sb = wp.tile([128, HS], bf16, tag="bb0s", name="bb0s")
            nc.sync.dma_start(bb0_sb[0:1, :], bb0[:])
            ba1_sb = wp.tile([128, 3 * HS], bf16, tag="ba1s", name="ba1s")
            nc.sync.dma_start(ba1_sb[0:1, :], ba1[:])
            bb1_sb = wp.tile([128, HS], bf16, tag="bb1s", name="bb1s")
            nc.sync.dma_start(bb1_sb[0:1, :], bb1[:])
            b1_sb = wp.tile([128, 8], fp32, tag="b1s", name="b1s")
            nc.sync.dma_start(b1_sb[:], b1c[:])
            b2_sb = wp.tile([128, 4], fp32, tag="b2s", name="b2s")
            nc.sync.dma_start(b2_sb[:], b2c[:])
            bgen_sb = wp.tile([128, VS], bf16, tag="bgens", name="bgens")
            nc.sync.dma_start(bgen_sb[0:1, :], bgen[:])
            ones_sb = wp.tile([128, 128], bf16, tag="ones", name="ones")
            nc.vector.memset(ones_sb[:], 1.0)

            # ---- state ----
            XT = st.tile([128, 4 * (T * B)], bf16, tag="XT", name="XT")        # x^T, 4 E-tiles
            ring0 = st.tile([128, 8 * RW], bf16, tag="ring0", name="ring0")    # h0 ring, 8 k-tiles
            H1T = st.tile([128, 8 * (16 * (T + 1))], bf16, tag="H1T", name="H1T")
            h_own0 = [st.tile([128, B], fp32, tag=f"ho0_{p}", name=f"ho0_{p}") for p in range(2)]
            h_own1 = [st.tile([128, B], fp32, tag=f"ho1_{p}", name=f"ho1_{p}") for p in range(2)]
            HW1 = 16 * (T + 1)

            # init h state
            nc.sync.dma_start(
                ring0[:].rearrange("p (k c) -> p k c", k=8)[:, :, 0:16],
                h0f[:].rearrange("(k p) c -> p k c", p=128))
            nc.sync.dma_start(
                H1T[:].rearrange("p (k c) -> p k c", k=8)[:, :, 0:16],
                h1f[:].rearrange("(k p) c -> p k c", p=128))
            nc.sync.dma_start(h_own0[0][:], h0o[:])
            nc.sync.dma_start(h_own1[0][:], h1o[:])

            # ---- P1: gather + transpose x ----
            with (
                tc.tile_pool(name="gp", bufs=2) as gp,
                tc.tile_pool(name="gps", bufs=2, space="PSUM") as gpp,
            ):
                idx_sb = gp.tile([128, NT], i32, tag="idx", name="idx")
                nc.sync.dma_start(idx_sb[:], idx[:].rearrange("a b -> b a"))
                ident = gp.tile([128, 128], fp32, tag="ident", name="ident")
                make_identity(nc, ident[:])
                for i in range(NT):
                    xg = gp.tile([128, E], fp32, tag="xg", name=f"xg{i}", bufs=2)
                    nc.gpsimd.indirect_dma_start(
                        out=xg[:], out_offset=None, in_=emb[:],
                        in_offset=bass.IndirectOffsetOnAxis(ap=idx_sb[:, i:i + 1], axis=0))
                    for e in range(4):
                        pt = gpp.tile([128, 128], fp32, tag="pt", name=f"pt{i}_{e}")
                        nc.tensor.transpose(pt[:], xg[:, 128 * e:128 * (e + 1)], ident[:])
                        nc.vector.tensor_copy(
                            XT[:, (T * B) * e + 128 * i: (T * B) * e + 128 * (i + 1)], pt[:])

            # ---- P2: recurrence supersteps ----
            def gate_groups(ps, whh_sb, wih_sb, kdim_i, rhs_h_fn, rhs_x_fn, baS, bbS):
                # build r,z,i_n,h_n psum groups into ps(128,64)
                for gi, (c0, gc) in enumerate([(0, 0), (16, HS), (32, 2 * HS)]):
                    # r (c0=0), z (c0=16): h-part + x-part + bias; i_n (c0=32): x-part + bias
                    mms = []
                    if gi < 2:
                        for k in range(8):
                            mms.append((whh_sb[:, 3 * HS * k + gc:3 * HS * k + gc + 128], rhs_h_fn(k)))
                    for k in range(kdim_i):
                        mms.append((wih_sb[:, 3 * HS * k + gc:3 * HS * k + gc + 128], rhs_x_fn(k)))
                    mms.append((baS[0:1, gc:gc + 128], ones_sb[0:1, 0:B]))
                    for mi, (lt, rr) in enumerate(mms):
                        nc.tensor.matmul(ps[:, c0:c0 + 16], lt, rr,
                                         start=(mi == 0), stop=(mi == len(mms) - 1))
                # h_n group (cols 48:64)
                mms = [(whh_sb[:, 3 * HS * k + 2 * HS:3 * HS * k + 2 * HS + 128], rhs_h_fn(k))
                       for k in range(8)]
                mms.append((bbS[0:1, 0:HS], ones_sb[0:1, 0:B]))
                for mi, (lt, rr) in enumerate(mms):
                    nc.tensor.matmul(ps[:, 48:64], lt, rr,
                                     start=(mi == 0), stop=(mi == len(mms) - 1))

            def gates(ps, hprev, hnew, cc_dst, tag, t):
                rz = st.tile([128, 32], fp32, tag=f"rz{tag}", name=f"rz{tag}_{t}", bufs=2)
                nc.scalar.activation(rz[:], ps[:, 0:32], mybir.ActivationFunctionType.Sigmoid)
                t1 = st.tile([128, B], fp32, tag=f"t1{tag}", name=f"t1{tag}_{t}", bufs=2)
                nc.vector.tensor_tensor(out=t1[:], in0=ps[:, 48:64], in1=rz[:, 0:16],
                                        op=mybir.AluOpType.mult)
                nc.vector.tensor_tensor(out=t1[:], in0=t1[:], in1=ps[:, 32:48],
                                        op=mybir.AluOpType.add)
                nsb = st.tile([128, B], fp32, tag=f"n{tag}", name=f"n{tag}_{t}", bufs=2)
                nc.scalar.activation(nsb[:], t1[:], mybir.ActivationFunctionType.Tanh)
                a = st.tile([128, B], fp32, tag=f"a{tag}", name=f"a{tag}_{t}", bufs=2)
                nc.vector.tensor_tensor(out=a[:], in0=hprev[:], in1=nsb[:],
                                        op=mybir.AluOpType.subtract)
                nc.vector.tensor_tensor(out=a[:], in0=a[:], in1=rz[:, 16:32],
                                        op=mybir.AluOpType.mult)
                nc.vector.tensor_tensor(out=hnew[:], in0=a[:], in1=nsb[:],
                                        op=mybir.AluOpType.add)
                nc.vector.tensor_copy(cc_dst, hnew[:])

            ring3 = ring0[:].rearrange("p (k c) -> p k c", k=8)
            h1t3 = H1T[:].rearrange("p (k c) -> p k c", k=8)

            psp_cm = tc.tile_pool(name="ps", bufs=2, space="PSUM")
            psp = psp_cm.__enter__()
            mp_cm = tc.tile_pool(name="mp", bufs=1)
            mp = mp_cm.__enter__()
            mpp_cm = tc.tile_pool(name="mps", bufs=1, space="PSUM")
            mpp = mpp_cm.__enter__()

            cc_srcs = {t: st.tile([128, 32], bf16, tag="cc_src", name=f"cc_src{t}", bufs=3)
                       for t in range(1, T + 3)}
            nc.vector.memset(cc_srcs[1][:, 16:32], 0.0)
            nc.vector.memset(cc_srcs[2][:, 16:32], 0.0)
            nc.vector.memset(cc_srcs[T + 1][:, 0:16], 0.0)
            nc.vector.memset(cc_srcs[T + 2][:, 0:16], 0.0)

            GC = 512 if T * B >= 512 else T * B
            NG = (T * B) // GC

            def p5_group(g):
                # tb rows [GC*g : GC*(g+1)] -> RT chunk, OUT chunk, logit M-tiles
                RTg = mp.tile([128, 8 * GC], bf16, tag="RTg", name=f"RTg{g}", bufs=2)
                for m in range(8):
                    pr = mpp.tile([128, GC], fp32, tag="pr", name=f"pr{g}_{m}", bufs=1)
                    for k in range(8):
                        nc.tensor.matmul(
                            pr[:], w1t_sb[:, H * k + 128 * m:H * k + 128 * (m + 1)],
                            H1T[:, HW1 * k + 16 + GC * g:HW1 * k + 16 + GC * (g + 1)],
                            start=(k == 0), stop=(k == 7))
                    nc.scalar.activation(
                        RTg[:, GC * m:GC * (m + 1)], pr[:],
                        mybir.ActivationFunctionType.Relu, bias=b1_sb[:, m:m + 1], scale=1.0)
                OUTg = mp.tile([128, 4 * GC], bf16, tag="OUTg", name=f"OUTg{g}", bufs=2)
                for m in range(4):
                    po = mpp.tile([128, GC], fp32, tag="po", name=f"po{g}_{m}", bufs=1)
                    for k in range(8):
                        nc.tensor.matmul(
                            po[:], w2t_sb[:, E * k + 128 * m:E * k + 128 * (m + 1)],
                            RTg[:, GC * k:GC * (k + 1)],
                            start=(k == 0), stop=(k == 7))
                    nc.scalar.activation(
                        OUTg[:, GC * m:GC * (m + 1)], po[:],
                        mybir.ActivationFunctionType.Identity, bias=b2_sb[:, m:m + 1], scale=1.0)
                for jj in range(GC // 128):
                    j = (GC // 128) * g + jj
                    lsb = mp.tile([128, VS], fp32, tag="lsb", name=f"lsb{j}", bufs=1)
                    for nchs in range(0, VS, 500):
                        pl = mpp.tile([128, 500], fp32, tag="pl", name=f"pl{j}_{nchs}", bufs=2)
                        for e in range(4):
                            nc.tensor.matmul(
                                pl[:], OUTg[:, GC * e + 128 * jj:GC * e + 128 * (jj + 1)],
                                embts_sb[:, VS * e + nchs:VS * e + nchs + 500],
                                start=(e == 0), stop=False)
                        nc.tensor.matmul(
                            pl[:], ones_sb[0:1, :], bgen_sb[0:1, nchs:nchs + 500],
                            start=False, stop=True)
                        nc.vector.tensor_copy(lsb[:, nchs:nchs + 500], pl[:])
                    nc.sync.dma_start(out[128 * j:128 * (j + 1), :], lsb[:])

            for t in range(1, T + 3):
                cc_src = cc_srcs[t]
                if t <= T:
                    # layer 0: h0_t  (critical chain)
                    ps0 = psp.tile([128, 64], fp32, tag="ps0", name=f"ps0_{t}", bufs=2)
                    so = 16 * ((t - 1) % 4)
                    gate_groups(
                        ps0, whh0_sb, wih0_sb, 4,
                        lambda k: ring0[:, RW * k + so:RW * k + so + 16],
                        lambda e: XT[:, (T * B) * e + B * (t - 1):(T * B) * e + B * t],
                        ba0_sb, bb0_sb)
                    gates(ps0, h_own0[(t - 1) % 2], h_own0[t % 2], cc_src[:, 0:16], "L0", t)

                ccin = dr.tile([128, 32], bf16, tag="ccin", name=f"ccin{t}", bufs=2)
                ccout = dr.tile([128 * N, 32], bf16, tag="ccout", name=f"ccout{t}",
                                bufs=2, addr_space="Shared")
                nc.sync.dma_start(ccin[:], cc_src[:])
                nc.gpsimd.collective_compute(
                    "AllGather", mybir.AluOpType.bypass,
                    replica_groups=[list(range(N))],
                    ins=[ccin[:]], outs=[ccout[:]])
                if t <= T:
                    nc.sync.dma_start(
                        ring3[:, :, 16 * (t % 4):16 * (t % 4) + 16],
                        ccout[:, 0:16].rearrange("(k p) c -> p k c", p=128))
                if t >= 3:
                    nc.sync.dma_start(
                        h1t3[:, :, 16 * (t - 2):16 * (t - 1)],
                        ccout[:, 16:32].rearrange("(k p) c -> p k c", p=128))

                if 2 <= t <= T + 1:
                    # layer 1: h1_{t-1}, off the critical chain (rides AG t+1)
                    ps1 = psp.tile([128, 64], fp32, tag="ps1", name=f"ps1_{t}", bufs=2)
                    so1 = 16 * ((t - 1) % 4)
                    gate_groups(
                        ps1, whh1_sb, wih1_sb, 8,
                        lambda k: H1T[:, HW1 * k + 16 * (t - 2):HW1 * k + 16 * (t - 1)],
                        lambda k: ring0[:, RW * k + so1:RW * k + so1 + 16],
                        ba1_sb, bb1_sb)
                    gates(ps1, h_own1[(t - 2) % 2], h_own1[(t - 1) % 2],
                          cc_srcs[t + 1][:, 16:32], "L1", t)

                # interleave vocab-projection work once its h1 block landed
                if T == S and t >= 34 and (t - 34) % 32 == 0 and (t - 34) // 32 < 3:
                    p5_group((t - 34) // 32)
            if T == S:
                p5_group(3)
            else:
                for g in range(NG):
                    p5_group(g)

            mpp_cm.__exit__(None, None, None)
            mp_cm.__exit__(None, None, None)
            psp_cm.__exit__(None, None, None)

    nc.finalize()
    return nc


def _prep_inputs(hidden, trg, embedding, w_ih0, w_hh0, b_ih0, b_hh0,
                 w_ih1, w_hh1, b_ih1, b_hh1, w1, b1, w2, b2, b_gen):
    bf = ml_dtypes.bfloat16
    T = STEPS
    f32 = np.float32
    hidden = np.asarray(hidden, f32)
    trg = np.asarray(trg)
    embedding = np.asarray(embedding, f32)
    in_maps = []
    # (t,b) index order
    idx_full = np.asarray(trg.T[:T], np.int32).reshape(-1)          # (T*B,)
    idx_tiles = idx_full.reshape(-1, 128).astype(np.int32)          # (NT,128)

    def gslice(wT, r):
        # wT (K, 3H) -> (K, 384) slice of each gate for core r
        cols = np.concatenate([np.arange(HS) + g * H + r * HS for g in range(3)])
        return np.ascontiguousarray(wT[:, cols])

    for r in range(N):
        sl = slice(r * HS, (r + 1) * HS)
        ba0_ = (b_ih0 + b_hh0).astype(f32)
        ba0v = np.concatenate([ba0_[0 * H + r * HS:0 * H + (r + 1) * HS],
                               ba0_[1 * H + r * HS:1 * H + (r + 1) * HS],
                               np.asarray(b_ih0, f32)[2 * H + r * HS:2 * H + (r + 1) * HS]])
        bb0v = np.asarray(b_hh0, f32)[2 * H + r * HS:2 * H + (r + 1) * HS]
        ba1_ = (b_ih1 + b_hh1).astype(f32)
        ba1v = np.concatenate([ba1_[0 * H + r * HS:0 * H + (r + 1) * HS],
                               ba1_[1 * H + r * HS:1 * H + (r + 1) * HS],
                               np.asarray(b_ih1, f32)[2 * H + r * HS:2 * H + (r + 1) * HS]])
        bb1v = np.asarray(b_hh1, f32)[2 * H + r * HS:2 * H + (r + 1) * HS]
        in_maps.append({
            "emb": embedding,
            "idx": idx_tiles,
            "h0f": hidden[0].T.astype(bf),
            "h1f": hidden[1].T.astype(bf),
            "h0o": np.ascontiguousarray(hidden[0].T[sl]).astype(f32),
            "h1o": np.ascontiguousarray(hidden[1].T[sl]).astype(f32),
            "wih0": gslice(np.asarray(w_ih0, f32).T, r).astype(bf),
            "whh0": gslice(np.asarray(w_hh0, f32).T, r).astype(bf),
            "wih1": gslice(np.asarray(w_ih1, f32).T, r).astype(bf),
            "whh1": gslice(np.asarray(w_hh1, f32).T, r).astype(bf),
            "ba0": ba0v.reshape(1, -1).astype(bf),
            "bb0": bb0v.reshape(1, -1).astype(bf),
            "ba1": ba1v.reshape(1, -1).astype(bf),
            "bb1": bb1v.reshape(1, -1).astype(bf),
            "w1t": np.asarray(w1, f32).T.astype(bf),
            "b1c": np.asarray(b1, f32).reshape(8, 128).T.astype(f32),
            "w2t": np.asarray(w2, f32).T.astype(bf),
            "b2c": np.asarray(b2, f32).reshape(4, 128).T.astype(f32),
            "embts": np.ascontiguousarray(embedding.T[:, r * VS:(r + 1) * VS]).astype(bf),
            "bgen": np.asarray(b_gen, f32)[r * VS:(r + 1) * VS].reshape(1, -1).astype(bf),
        })
    return in_maps


def kernel(**inputs):
    from concourse.bass_utils import run_bass_kernel_spmd
    if "nc" not in _cache:
        _cache["nc"] = _build()
    nc = _cache["nc"]
    in_maps = _prep_inputs(**inputs)
    res = run_bass_kernel_spmd(nc, in_maps, core_ids=list(range(N)))
    T = STEPS
    outf = np.empty((B, T, V), np.float32)
    for r in range(N):
        lr = res.results[r]["out"].reshape(T, B, VS)
        outf[:, :, r * VS:(r + 1) * VS] = lr.transpose(1, 0, 2)
    return outf


# revision 8
# speedup vs baseline: 1.0419x; 1.0302x over previous
"""Trainium2 Bass kernel for nn_Decoder (2-layer GRU decoder, weight-tied vocab projection).

Strategy (8 NeuronCores, SPMD):
  - Tensor-parallel recurrence: each core computes a 128-row slice of every GRU
    gate (H=1024 -> 8 x 128). Per superstep, one fused AllGather exchanges the
    new h0/h1 slices (bf16) across cores.
  - Gate preactivations are built entirely in PSUM by fused matmul groups
    (w_hh @ h  +  w_ih @ x  + bias-outer-product), fp32 accumulate.
  - The h-state for the "z*h_prev" path is kept in fp32 locally (h_own); only
    matmul operands are bf16.
  - Vocab-tied projection: embedding^T is sharded 4000 cols/core; the MLP is
    computed replicated (cheap) and logits are vocab-sharded.
Output: (B=16, S=128, V=32000) fp32, assembled host-side.
"""
import os
import numpy as np
import ml_dtypes

V, E, H, L = 32000, 512, 1024, 2
B, S = 16, 128
N = 8                  # cores
HS = H // N            # 128 rows of h per core
VS = V // N            # 4000 vocab cols per core
TB = B * S             # 2048 (t,b) rows
STEPS = int(os.environ.get("K_STEPS", str(S)))  # reduced for smoke testing

_cache = {}


def _build():
    import concourse.bass as bass
    import concourse.bacc as bacc
    import concourse.mybir as mybir
    import concourse.tile as tile
    from concourse.masks import make_identity

    fp32 = mybir.dt.float32
    bf16 = mybir.dt.bfloat16
    i32 = mybir.dt.int32
    T = STEPS
    NT = TB // 128 if T == S else (T * B) // 128   # number of 128-row (t,b) tiles
    RW = 64            # h0 ring: 4 slots x 16 cols per k-tile

    nc = bacc.Bacc("TRN2", num_devices=N, target_bir_lowering=False)

    # ---- DRAM I/O ----
    emb = nc.dram_tensor("emb", [V, E], fp32, kind="ExternalInput")
    idx = nc.dram_tensor("idx", [B * T // 128 * 8 if False else (T * B // 128), 128], i32, kind="ExternalInput")
    h0f = nc.dram_tensor("h0f", [H, B], bf16, kind="ExternalInput")
    h1f = nc.dram_tensor("h1f", [H, B], bf16, kind="ExternalInput")
    h0o = nc.dram_tensor("h0o", [HS, B], fp32, kind="ExternalInput")
    h1o = nc.dram_tensor("h1o", [HS, B], fp32, kind="ExternalInput")
    wih0 = nc.dram_tensor("wih0", [E, 3 * HS], bf16, kind="ExternalInput")
    whh0 = nc.dram_tensor("whh0", [H, 3 * HS], bf16, kind="ExternalInput")
    wih1 = nc.dram_tensor("wih1", [H, 3 * HS], bf16, kind="ExternalInput")
    whh1 = nc.dram_tensor("whh1", [H, 3 * HS], bf16, kind="ExternalInput")
    ba0 = nc.dram_tensor("ba0", [1, 3 * HS], bf16, kind="ExternalInput")
    bb0 = nc.dram_tensor("bb0", [1, HS], bf16, kind="ExternalInput")
    ba1 = nc.dram_tensor("ba1", [1, 3 * HS], bf16, kind="ExternalInput")
    bb1 = nc.dram_tensor("bb1", [1, HS], bf16, kind="ExternalInput")
    w1t = nc.dram_tensor("w1t", [H, H], bf16, kind="ExternalInput")
    b1c = nc.dram_tensor("b1c", [128, 8], fp32, kind="ExternalInput")
    w2t = nc.dram_tensor("w2t", [H, E], bf16, kind="ExternalInput")
    b2c = nc.dram_tensor("b2c", [128, 4], fp32, kind="ExternalInput")
    embts = nc.dram_tensor("embts", [E, VS], bf16, kind="ExternalInput")
    bgen = nc.dram_tensor("bgen", [1, VS], bf16, kind="ExternalInput")
    out = nc.dram_tensor("out", [T * B, VS], fp32, kind="ExternalOutput")

    with tile.TileContext(nc) as tc:
        with (
            tc.tile_pool(name="wp", bufs=1) as wp,
            tc.tile_pool(name="state", bufs=1) as st,
            tc.tile_pool(name="dram", bufs=4, space="DRAM") as dr,
        ):
            # ---- resident weights ----
            def load3d(name, src, kdim, cols):
                # src (kdim*128, cols) -> sbuf (128, kdim*cols), k-major blocks
                tl = wp.tile([128, kdim * cols], bf16, tag=name, name=name)
                nc.sync.dma_start(
                    tl[:].rearrange("p (k c) -> p k c", k=kdim),
                    src[:].rearrange("(k p) c -> p k c", p=128))
                return tl

            wih0_sb = load3d("wih0_sb", wih0, 4, 3 * HS)
            whh0_sb = load3d("whh0_sb", whh0, 8, 3 * HS)
            wih1_sb = load3d("wih1_sb", wih1, 8, 3 * HS)
            whh1_sb = load3d("whh1_sb", whh1, 8, 3 * HS)
            w1t_sb = load3d("w1t_sb", w1t, 8, H)
            w2t_sb = load3d("w2t_sb", w2t, 8, E)
            embts_sb = load3d("embts_sb", embts, 4, VS)
            ba0_sb = wp.tile([128, 3 * HS], bf16, tag="ba0s", name="ba0s")
            nc.sync.dma_start(ba0_sb[0:1, :], ba0[:])
            bb0_sb = wp.tile([128, HS], bf16, tag="bb0s", name="bb0s")
            nc.sync.dma_start(bb0_sb[0:1, :], bb0[:])
            ba1_sb = wp.tile([128, 3 * HS], bf16, tag="ba1s", name="ba1s")
            nc.sync.dma_start(ba1_sb[0:1, :], ba1[:])
            bb1_sb = wp.tile([128, HS], bf16, tag="bb1s", name="bb1s")
            nc.sync.dma_start(bb1_sb[0:1, :], bb1[:])
            b1_sb = wp.tile([128, 8], fp32, tag="b1s", name="b1s")
            nc.sync.dma_start(b1_sb[:], b1c[:])
            b2_sb = wp.tile([128, 4], fp32, tag="b2s", name="b2s")
            nc.sync.dma_start(b2_sb[:], b2c[:])
            bgen_sb = wp.tile([128, VS], bf16, tag="bgens", name="bgens")
            nc.sync.dma_start(bgen_sb[0:1, :], bgen[:])
            ones_sb = wp.tile([128, 128], bf16, tag="ones", name="ones")
            nc.vector.memset(ones_sb[:], 1.0)

            # ---- state ----
            XT = st.tile([128, 4 * (T * B)], bf16, tag="XT", name="XT")        # x^T, 4 E-tiles
            ring0 = st.tile([128, 8 * RW], bf16, tag="ring0", name="ring0")    # h0 ring, 8 k-tiles
            H1T = st.tile([128, 8 * (16 * (T + 1))], bf16, tag="H1T", name="H1T")
            h_own0 = [st.tile([128, B], fp32, tag=f"ho0_{p}", name=f"ho0_{p}") for p in range(2)]
            h_own1 = [st.tile([128, B], fp32, tag=f"ho1_{p}", name=f"ho1_{p}") for p in range(2)]
            HW1 = 16 * (T + 1)

            # init h state
            nc.sync.dma_start(
                ring0[:].rearrange("p (k c) -> p k c", k=8)[:, :, 0:16],
                h0f[:].rearrange("(k p) c -> p k c", p=128))
            nc.sync.dma_start(
                H1T[:].rearrange("p (k c) -> p k c", k=8)[:, :, 0:16],
                h1f[:].rearrange("(k p) c -> p k c", p=128))
            nc.sync.dma_start(h_own0[0][:], h0o[:])
            nc.sync.dma_start(h_own1[0][:], h1o[:])

            # ---- P1: gather + transpose x ----
            with (
                tc.tile_pool(name="gp", bufs=2) as gp,
                tc.tile_pool(name="gps", bufs=2, space="PSUM") as gpp,
            ):
                idx_sb = gp.tile([128, NT], i32, tag="idx", name="idx")
                nc.sync.dma_start(idx_sb[:], idx[:].rearrange("a b -> b a"))
                ident = gp.tile([128, 128], fp32, tag="ident", name="ident")
                make_identity(nc, ident[:])
                for i in range(NT):
                    xg = gp.tile([128, E], fp32, tag="xg", name=f"xg{i}", bufs=2)
                    nc.gpsimd.indirect_dma_start(
                        out=xg[:], out_offset=None, in_=emb[:],
                        in_offset=bass.IndirectOffsetOnAxis(ap=idx_sb[:, i:i + 1], axis=0))
                    for e in range(4):
                        pt = gpp.tile([128, 128], fp32, tag="pt", name=f"pt{i}_{e}")
                        nc.tensor.transpose(pt[:], xg[:, 128 * e:128 * (e + 1)], ident[:])
                        nc.vector.tensor_copy(
                            XT[:, (T * B) * e + 128 * i: (T * B) * e + 128 * (i + 1)], pt[:])

            # ---- P2: recurrence supersteps ----
            def gate_groups(ps, whh_sb, wih_sb, kdim_i, rhs_h_fn, rhs_x_fn, baS, bbS):
                # build r,z,i_n,h_n psum groups into ps(128,64)
                for gi, (c0, gc) in enumerate([(0, 0), (16, HS), (32, 2 * HS)]):
                    # r (c0=0), z (c0=16): h-part + x-part + bias; i_n (c0=32): x-part + bias
                    mms = []
                    if gi < 2:
                        for k in range(8):
                            mms.append((whh_sb[:, 3 * HS * k + gc:3 * HS * k + gc + 128], rhs_h_fn(k)))
                    for k in range(kdim_i):
                        mms.append((wih_sb[:, 3 * HS * k + gc:3 * HS * k + gc + 128], rhs_x_fn(k)))
                    mms.append((baS[0:1, gc:gc + 128], ones_sb[0:1, 0:B]))
                    for mi, (lt, rr) in enumerate(mms):
                        nc.tensor.matmul(ps[:, c0:c0 + 16], lt, rr,
                                         start=(mi == 0), stop=(mi == len(mms) - 1))
                # h_n group (cols 48:64)
                mms = [(whh_sb[:, 3 * HS * k + 2 * HS:3 * HS * k + 2 * HS + 128], rhs_h_fn(k))
                       for k in range(8)]
                mms.append((bbS[0:1, 0:HS], ones_sb[0:1, 0:B]))
                for mi, (lt, rr) in enumerate(mms):
                    nc.tensor.matmul(ps[:, 48:64], lt, rr,
                                     start=(mi == 0), stop=(mi == len(mms) - 1))

            def gates(ps, hprev, hnew, cc_dst, tag, t):
                rz = st.tile([128, 32], fp32, tag=f"rz{tag}", name=f"rz{tag}_{t}", bufs=2)
                nc.scalar.activation(rz[:], ps[:, 0:32], mybir.ActivationFunctionType.Sigmoid)
                t1 = st.tile([128, B], fp32, tag=f"t1{tag}", name=f"t1{tag}_{t}", bufs=2)
                nc.vector.tensor_tensor(out=t1[:], in0=ps[:, 48:64], in1=rz[:, 0:16],
                                        op=mybir.AluOpType.mult)
                nc.vector.tensor_tensor(out=t1[:], in0=t1[:], in1=ps[:, 32:48],
                                        op=mybir.AluOpType.add)
                nsb = st.tile([128, B], fp32, tag=f"n{tag}", name=f"n{tag}_{t}", bufs=2)
                nc.scalar.activation(nsb[:], t1[:], mybir.ActivationFunctionType.Tanh)
                a = st.tile([128, B], fp32, tag=f"a{tag}", name=f"a{tag}_{t}", bufs=2)
                nc.vector.tensor_tensor(out=a[:], in0=hprev[:], in1=nsb[:],
                                        op=mybir.AluOpType.subtract)
                nc.vector.tensor_tensor(out=a[:], in0=a[:], in1=rz[:, 16:32],
                                        op=mybir.AluOpType.mult)
                nc.vector.tensor_tensor(out=hnew[:], in0=a[:], in1=nsb[:],
                                        op=mybir.AluOpType.add)
                nc.vector.tensor_copy(cc_dst, hnew[:])

            ring3 = ring0[:].rearrange("p (k c) -> p k c", k=8)
            h1t3 = H1T[:].rearrange("p (k c) -> p k c", k=8)

            psp_cm = tc.tile_pool(name="ps", bufs=2, space="PSUM")
            psp = psp_cm.__enter__()
            mp_cm = tc.tile_pool(name="mp", bufs=1)
            mp = mp_cm.__enter__()
            mpp_cm = tc.tile_pool(name="mps", bufs=1, space="PSUM")
            mpp = mpp_cm.__enter__()

            cc_srcs = {t: st.tile([128, 32], bf16, tag="cc_src", name=f"cc_src{t}", bufs=3)
                       for t in range(1, T + 3)}
            nc.vector.memset(cc_srcs[1][:, 16:32], 0.0)
            nc.vector.memset(cc_srcs[2][:, 16:32], 0.0)
            nc.vector.memset(cc_srcs[T + 1][:, 0:16], 0.0)
            nc.vector.memset(cc_srcs[T + 2][:, 0:16], 0.0)

            GC = 128
            NG = (T * B) // GC

            def p5_group(g):
                # tb rows [GC*g : GC*(g+1)] -> RT chunk, OUT chunk, logit M-tiles
                RTg = mp.tile([128, 8 * GC], bf16, tag="RTg", name=f"RTg{g}", bufs=2)
                for m in range(8):
                    pr = mpp.tile([128, GC], fp32, tag="pr", name=f"pr{g}_{m}", bufs=1)
                    for k in range(8):
                        nc.tensor.matmul(
                            pr[:], w1t_sb[:, H * k + 128 * m:H * k + 128 * (m + 1)],
                            H1T[:, HW1 * k + 16 + GC * g:HW1 * k + 16 + GC * (g + 1)],
                            start=(k == 0), stop=(k == 7))
                    nc.scalar.activation(
                        RTg[:, GC * m:GC * (m + 1)], pr[:],
                        mybir.ActivationFunctionType.Relu, bias=b1_sb[:, m:m + 1], scale=1.0)
                OUTg = mp.tile([128, 4 * GC], bf16, tag="OUTg", name=f"OUTg{g}", bufs=2)
                for m in range(4):
                    po = mpp.tile([128, GC], fp32, tag="po", name=f"po{g}_{m}", bufs=1)
                    for k in range(8):
                        nc.tensor.matmul(
                            po[:], w2t_sb[:, E * k + 128 * m:E * k + 128 * (m + 1)],
                            RTg[:, GC * k:GC * (k + 1)],
                            start=(k == 0), stop=(k == 7))
                    nc.scalar.activation(
                        OUTg[:, GC * m:GC * (m + 1)], po[:],
                        mybir.ActivationFunctionType.Identity, bias=b2_sb[:, m:m + 1], scale=1.0)
                for jj in range(GC // 128):
                    j = (GC // 128) * g + jj
                    lsb = mp.tile([128, VS], fp32, tag="lsb", name=f"lsb{j}", bufs=1)
                    for nchs in range(0, VS, 500):
                        pl = mpp.tile([128, 500], fp32, tag="pl", name=f"pl{j}_{nchs}", bufs=2)
                        for e in range(4):
                            nc.tensor.matmul(
                                pl[:], OUTg[:, GC * e + 128 * jj:GC * e + 128 * (jj + 1)],
                                embts_sb[:, VS * e + nchs:VS * e + nchs + 500],
                                start=(e == 0), stop=False)
                        nc.tensor.matmul(
                            pl[:], ones_sb[0:1, :], bgen_sb[0:1, nchs:nchs + 500],
                            start=False, stop=True)
                        nc.vector.tensor_copy(lsb[:, nchs:nchs + 500], pl[:])
                    nc.gpsimd.dma_start(out[128 * j:128 * (j + 1), 0:VS // 2], lsb[:, 0:VS // 2])
                    nc.gpsimd.dma_start(out[128 * j:128 * (j + 1), VS // 2:VS], lsb[:, VS // 2:VS])

            for t in range(1, T + 3):
                cc_src = cc_srcs[t]
                if t <= T:
                    # layer 0: h0_t  (critical chain)
                    ps0 = psp.tile([128, 64], fp32, tag="ps0", name=f"ps0_{t}", bufs=2)
                    so = 16 * ((t - 1) % 4)
                    gate_groups(
                        ps0, whh0_sb, wih0_sb, 4,
                        lambda k: ring0[:, RW * k + so:RW * k + so + 16],
                        lambda e: XT[:, (T * B) * e + B * (t - 1):(T * B) * e + B * t],
                        ba0_sb, bb0_sb)
                    gates(ps0, h_own0[(t - 1) % 2], h_own0[t % 2], cc_src[:, 0:16], "L0", t)

                ccin = dr.tile([128, 32], bf16, tag="ccin", name=f"ccin{t}", bufs=2)
                ccout = dr.tile([128 * N, 32], bf16, tag="ccout", name=f"ccout{t}",
                                bufs=2, addr_space="Shared")
                nc.sync.dma_start(ccin[:], cc_src[:])
                nc.gpsimd.collective_compute(
                    "AllGather", mybir.AluOpType.bypass,
                    replica_groups=[list(range(N))],
                    ins=[ccin[:]], outs=[ccout[:]])
                if t <= T:
                    nc.sync.dma_start(
                        ring3[:, :, 16 * (t % 4):16 * (t % 4) + 16],
                        ccout[:, 0:16].rearrange("(k p) c -> p k c", p=128))
                if t >= 3:
                    nc.sync.dma_start(
                        h1t3[:, :, 16 * (t - 2):16 * (t - 1)],
                        ccout[:, 16:32].rearrange("(k p) c -> p k c", p=128))

                if 2 <= t <= T + 1:
                    # layer 1: h1_{t-1}, off the critical chain (rides AG t+1)
                    ps1 = psp.tile([128, 64], fp32, tag="ps1", name=f"ps1_{t}", bufs=2)
                    so1 = 16 * ((t - 1) % 4)
                    gate_groups(
                        ps1, whh1_sb, wih1_sb, 8,
                        lambda k: H1T[:, HW1 * k + 16 * (t - 2):HW1 * k + 16 * (t - 1)],
                        lambda k: ring0[:, RW * k + so1:RW * k + so1 + 16],
                        ba1_sb, bb1_sb)
                    gates(ps1, h_own1[(t - 2) % 2], h_own1[(t - 1) % 2],
                          cc_srcs[t + 1][:, 16:32], "L1", t)

                # interleave vocab-projection work once its h1 block landed
                if T == S and t >= 10 and (t - 10) % 8 == 0 and (t - 10) // 8 < NG - 1:
                    p5_group((t - 10) // 8)
            if T == S:
                p5_group(NG - 1)
            else:
                for g in range(NG):
                    p5_group(g)

            mpp_cm.__exit__(None, None, None)
            mp_cm.__exit__(None, None, None)
            psp_cm.__exit__(None, None, None)

    nc.finalize()
    return nc


def _prep_inputs(hidden, trg, embedding, w_ih0, w_hh0, b_ih0, b_hh0,
                 w_ih1, w_hh1, b_ih1, b_hh1, w1, b1, w2, b2, b_gen):
    bf = ml_dtypes.bfloat16
    T = STEPS
    f32 = np.float32
    hidden = np.asarray(hidden, f32)
    trg = np.asarray(trg)
    embedding = np.asarray(embedding, f32)
    in_maps = []
    # (t,b) index order
    idx_full = np.asarray(trg.T[:T], np.int32).reshape(-1)          # (T*B,)
    idx_tiles = idx_full.reshape(-1, 128).astype(np.int32)          # (NT,128)

    def gslice(wT, r):
        # wT (K, 3H) -> (K, 384) slice of each gate for core r
        cols = np.concatenate([np.arange(HS) + g * H + r * HS for g in range(3)])
        return np.ascontiguousarray(wT[:, cols])

    for r in range(N):
        sl = slice(r * HS, (r + 1) * HS)
        ba0_ = (b_ih0 + b_hh0).astype(f32)
        ba0v = np.concatenate([ba0_[0 * H + r * HS:0 * H + (r + 1) * HS],
                               ba0_[1 * H + r * HS:1 * H + (r + 1) * HS],
                               np.asarray(b_ih0, f32)[2 * H + r * HS:2 * H + (r + 1) * HS]])
        bb0v = np.asarray(b_hh0, f32)[2 * H + r * HS:2 * H + (r + 1) * HS]
        ba1_ = (b_ih1 + b_hh1).astype(f32)
        ba1v = np.concatenate([ba1_[0 * H + r * HS:0 * H + (r + 1) * HS],
                               ba1_[1 * H + r * HS:1 * H + (r + 1) * HS],
                               np.asarray(b_ih1, f32)[2 * H + r * HS:2 * H + (r + 1) * HS]])
        bb1v = np.asarray(b_hh1, f32)[2 * H + r * HS:2 * H + (r + 1) * HS]
        in_maps.append({
            "emb": embedding,
            "idx": idx_tiles,
            "h0f": hidden[0].T.astype(bf),
            "h1f": hidden[1].T.astype(bf),
            "h0o": np.ascontiguousarray(hidden[0].T[sl]).astype(f32),
            "h1o": np.ascontiguousarray(hidden[1].T[sl]).astype(f32),
            "wih0": gslice(np.asarray(w_ih0, f32).T, r).astype(bf),
            "whh0": gslice(np.asarray(w_hh0, f32).T, r).astype(bf),
            "wih1": gslice(np.asarray(w_ih1, f32).T, r).astype(bf),
            "whh1": gslice(np.asarray(w_hh1, f32).T, r).astype(bf),
            "ba0": ba0v.reshape(1, -1).astype(bf),
            "bb0": bb0v.reshape(1, -1).astype(bf),
            "ba1": ba1v.reshape(1, -1).astype(bf),
            "bb1": bb1v.reshape(1, -1).astype(bf),
            "w1t": np.asarray(w1, f32).T.astype(bf),
            "b1c": np.asarray(b1, f32).reshape(8, 128).T.astype(f32),
            "w2t": np.asarray(w2, f32).T.astype(bf),
            "b2c": np.asarray(b2, f32).reshape(4, 128).T.astype(f32),
            "embts": np.ascontiguousarray(embedding.T[:, r * VS:(r + 1) * VS]).astype(bf),
            "bgen": np.asarray(b_gen, f32)[r * VS:(r + 1) * VS].reshape(1, -1).astype(bf),
        })
    return in_maps


def kernel(**inputs):
    from concourse.bass_utils import run_bass_kernel_spmd
    if "nc" not in _cache:
        _cache["nc"] = _build()
    nc = _cache["nc"]
    in_maps = _prep_inputs(**inputs)
    res = run_bass_kernel_spmd(nc, in_maps, core_ids=list(range(N)))
    T = STEPS
    outf = np.empty((B, T, V), np.float32)
    for r in range(N):
        lr = res.results[r]["out"].reshape(T, B, VS)
        outf[:, :, r * VS:(r + 1) * VS] = lr.transpose(1, 0, 2)
    return outf
